# revision 1
# baseline (speedup 1.0000x reference)
# Neural-collapse regularizer (tr_SW / tr_SB) on 8 TRN2 NeuronCores.
#
# Math: with per-class sums S_c = sum_{i: l_i=c} x_i, counts n_c,
# ssq = sum_i ||x_i||^2:
#   tr_SW = ssq - sum_c ||S_c||^2 / n_c
#   tr_SB = sum_c ||S_c/n_c - g||^2,  g = (sum_c S_c) / N
# The device computes the segment sums [128, D] per core plus ssq;
# everything else is tiny O(C*D) host math.
#
# Sharding: class-parallel. Core k owns classes [128k, 128(k+1)); the host
# routes each row to the core that owns its label.
#
# Layout: rows are packed in chunks of GRP=8 rows of a single class, one
# chunk per (group, partition) slot; all 8 row-tiles of a group share one
# one-hot. Features are fp8 (e4m3) to halve HBM traffic.
#
# Segment sums: DoubleRow fp8 matmuls — each MM consumes a PAIR of row
# tiles (contraction 2x128) against a duplicated one-hot OH2 [128, 2, 128],
# accumulating into one PSUM bank across all groups.
#
# ssq: per group's [128, 4096] fp8 buffer, columns are split three ways:
#   ACT:  Square + accum_out on cols [0, CA)
#   DVE:  scalar_tensor_tensor square + accum_out on cols [CA, CB)
#   PE:   "gram" matmuls B^T B on 128-col blocks of [CB, 4096), accumulated
#         into a [128,128] PSUM across the whole run; host takes the trace.
# Row partials land in a bf16 srow output; host sums them.

import contextlib
import ctypes
import os
import sys
import types

import numpy as np
import ml_dtypes

import concourse.bass as bass
import concourse.bacc as bacc
import concourse.mybir as mybir
from concourse.bass_utils import run_bass_kernel_spmd


def _ensure_ntff_hook():
    """Provide antenv.axon_hooks + an NTFF profile hook when the image's
    antenv package lacks it (needed only for trace=True timing runs)."""
    try:
        from antenv.axon_hooks import get_axon_ntff_profile_hook  # noqa: F401
        return
    except ImportError:
        pass
    mod = types.ModuleType("antenv.axon_hooks")
    state = {"hook": None}
    mod.set_axon_ntff_profile_hook = lambda h: state.__setitem__("hook", h)
    mod.get_axon_ntff_profile_hook = lambda: state["hook"]
    sys.modules["antenv.axon_hooks"] = mod

    so_path = "/opt/axon/libaxon_pjrt.so"
    if not os.path.exists(so_path):
        return
    lib = ctypes.CDLL(so_path)
    if not hasattr(lib, "axon_start_nrt_profile"):
        return
    lib.axon_start_nrt_profile.argtypes = [
        ctypes.POINTER(ctypes.c_int64), ctypes.c_size_t]
    lib.axon_start_nrt_profile.restype = ctypes.c_int64
    lib.axon_stop_nrt_profile.argtypes = [ctypes.c_char_p]
    lib.axon_stop_nrt_profile.restype = ctypes.c_int64

    @contextlib.contextmanager
    def _hook(output_dir, device_ids):
        import jax
        jax.devices()
        if device_ids:
            ids = (ctypes.c_int64 * len(device_ids))(*device_ids)
            rc = lib.axon_start_nrt_profile(ids, len(device_ids))
        else:
            rc = lib.axon_start_nrt_profile(None, 0)
        if rc != 0:
            raise RuntimeError(f"axon_start_nrt_profile rc={rc}")
        try:
            yield
        finally:
            n = lib.axon_stop_nrt_profile(str(output_dir).encode())
            print(f"profile: {n} file(s) written to {output_dir}",
                  file=sys.stderr)

    mod.set_axon_ntff_profile_hook(_hook)


CORES = 8
P = 128              # partitions = classes per core
D = 512              # feature dim
GRP = 8              # row-tiles per group = rows per chunk
GCOLS = GRP * D      # 4096 fp8 bytes per partition per group
FP8 = mybir.dt.float8e4
BF16 = mybir.dt.bfloat16
F32 = mybir.dt.float32
NP_FP8 = ml_dtypes.float8_e4m3fn
NP_BF16 = ml_dtypes.bfloat16

# Column split of each group's [128, 4096] buffer for the ssq work.
CA = int(os.environ.get("K_CA", "1408"))    # ACT: cols [0, CA)
CB = int(os.environ.get("K_CB", "2688"))    # DVE: cols [CA, CB)
CC = int(os.environ.get("K_CC", "2688"))    # GPSIMD: cols [CB, CC); PE: rest
WARM = int(os.environ.get("K_WARM", "22"))  # PE warm-up matmuls
XBD = int(os.environ.get("K_XBD", "6"))     # double-group x buffers
OHB = int(os.environ.get("K_OHB", "6"))     # one-hot buffers
USE_DR = os.environ.get("K_DR", "1") == "1"
DBG = set(os.environ.get("K_DBG", "").split(","))
assert CA % 128 == 0 and CB % 128 == 0 and CC % 128 == 0
assert CA <= CB <= CC <= GCOLS
NB = (GCOLS - CC) // 128                    # gram blocks per group


def _host_shard(features: np.ndarray, labels: np.ndarray):
    """Chunked class-sorted fp8 layout.

    Returns (in_maps, G). in_maps[k]:
      feat:  [G, 128, 4096] fp8 -- slot (g, p) holds GRP rows of one class
      lab:   [128, G] f32 -- rebased class (0..127) of slot (g, p)
      iota2: [128, 256] bf16 -- two copies of 0..127 per row
    """
    N, d = features.shape
    assert d == D, f"expected D={D}, got {d}"
    CPAD = CORES * P

    order = np.argsort(labels, kind="stable")
    sl = labels[order]
    class_start = np.searchsorted(sl, np.arange(CPAD + 1))
    counts = np.diff(class_start)
    chunks_per_class = -(-counts // GRP)
    core_chunks = chunks_per_class.reshape(CORES, P)
    G = int(-(-core_chunks.sum(axis=1).max() // P))

    f8 = features.astype(NP_FP8)
    io = np.arange(P, dtype=NP_BF16)
    iota2 = np.broadcast_to(np.concatenate([io, io]), (P, 2 * P)).copy()

    in_maps = []
    for k in range(CORES):
        nch = core_chunks[k]
        total = int(nch.sum())
        assert total <= G * P
        chunk_cls = np.repeat(np.arange(P), nch)
        grid = np.full((G * P, GRP), -1, dtype=np.int64)
        cls_pad_start = np.concatenate(([0], np.cumsum(nch * GRP)))
        cnts = counts[k * P:(k + 1) * P]
        lo = class_start[k * P]
        n_k = int(cnts.sum())
        rows_k = order[lo:lo + n_k]
        within = np.arange(n_k) - np.repeat(class_start[k * P:(k + 1) * P] - lo,
                                            cnts)
        pos = np.repeat(cls_pad_start[:-1], cnts) + within
        grid.reshape(-1)[pos] = rows_k

        safe = np.maximum(grid, 0)
        fr = f8[safe.reshape(-1)]
        fr[grid.reshape(-1) < 0] = 0
        feat = np.ascontiguousarray(fr.reshape(G, P, GCOLS))

        labg = np.zeros((G * P,), dtype=np.float32)
        labg[:total] = chunk_cls
        labg = np.ascontiguousarray(labg.reshape(G, P).T)

        in_maps.append({"feat": feat, "lab": labg, "iota2": iota2})
    return in_maps, G


def _build_raw(G: int):
    # DMA plan: first three groups land as singles (fast pipeline fill),
    # then pairs; each plan entry owns one rotating double-width slot.
    plan = []
    g0 = 0
    while g0 < G:
        cnt = 1 if (g0 < 3 or g0 >= G - 2) else min(2, G - g0)
        plan.append((g0, cnt))
        g0 += cnt
    NPLAN = len(plan)
    _g2idx = {}
    for _i, (_gs, _cnt) in enumerate(plan):
        for _j in range(_cnt):
            _g2idx[_gs + _j] = (_i, _j)

    nc = bacc.Bacc("TRN2", target_bir_lowering=False, debug=False,
                   enable_asserts=False)
    feat_h = nc.dram_tensor("feat", [G, P, GCOLS], FP8, kind="ExternalInput")
    lab_h = nc.dram_tensor("lab", [P, G], F32, kind="ExternalInput")
    iota2_h = nc.dram_tensor("iota2", [P, 2 * P], BF16, kind="ExternalInput")
    out_h = nc.dram_tensor("out", [P, D + P], F32, kind="ExternalOutput")
    srow_h = nc.dram_tensor("srow", [P, 3 * NPLAN], BF16,
                        kind="ExternalOutput")

    x_sb = nc.alloc_sbuf_tensor("x_sb", [P, XBD * 2 * GCOLS], FP8)
    oh_sb = nc.alloc_sbuf_tensor("oh_sb", [P, OHB * 2 * P], FP8)
    scr_d = nc.alloc_sbuf_tensor("scr_d",
                                 [P, 3 * 2 * max(CB - CA, 16)], FP8)
    scr_a = nc.alloc_sbuf_tensor("scr_a", [P, 3 * 2 * max(CA, 16)], FP8)
    scr_t = nc.alloc_sbuf_tensor("scr_t", [P, 16], FP8)
    scr_g = nc.alloc_sbuf_tensor("scr_g", [P, 3 * max(CC - CB, 16)], FP8)
    iota2_sb = nc.alloc_sbuf_tensor("iota2_sb", [P, 2 * P], BF16)
    lab_sb = nc.alloc_sbuf_tensor("lab_sb", [P, G], F32)
    srow_sb = nc.alloc_sbuf_tensor("srow_sb", [P, 3 * NPLAN], BF16)
    out_sb = nc.alloc_sbuf_tensor("out_sb", [P, D + P], F32)
    psum_s = nc.alloc_psum_tensor("psum_s", [P, D], F32)
    psum_g = nc.alloc_psum_tensor("psum_g", [P, P], F32)
    psum_w = nc.alloc_psum_tensor("psum_w", [P, P], F32)

    NDMA = NPLAN

    def plan_idx(g):
        return _g2idx[g][0]

    def ent_ap(i, cnt, lo, hi):
        # 3D slice [128, cnt, hi-lo] of plan entry i's slot columns
        base = (i % XBD) * 2 * GCOLS
        ap3 = x_sb.ap()[:, base:base + cnt * GCOLS].rearrange(
            "p (c w) -> p c w", c=cnt)
        return ap3[:, :, lo:hi]

    def xg_ap(g):
        i, sub = _g2idx[g]
        off = (i % XBD) * 2 * GCOLS + sub * GCOLS
        return x_sb.ap()[:, off:off + GCOLS]

    def oh_ap(g):
        return oh_sb.ap()[:, (g % OHB) * 2 * P:(g % OHB + 1) * 2 * P]

    import contextlib as _ctx
    with (
        _ctx.ExitStack() as _sems,
        nc.semaphore("sem_oh") as sem_oh,
        nc.semaphore("sem_sd") as sem_sd,
        nc.semaphore("sem_sa") as sem_sa,
        nc.semaphore("sem_pe") as sem_pe,
        nc.semaphore("sem_aux") as sem_aux,
        nc.semaphore("sem_lab") as sem_lab,
        nc.semaphore("sem_cp") as sem_cp,
        nc.semaphore("sem_seg") as sem_seg,
        nc.semaphore("sem_sg") as sem_sg,
        nc.semaphore("sem_out") as sem_out,
        nc.Block() as block,
    ):
        sem_xs = [_sems.enter_context(nc.semaphore(f"sem_x{b}"))
                  for b in range(XBD)]

        def wait_x(eng, g):
            i = plan_idx(g)
            eng.wait_ge(sem_xs[i % XBD], 16 * (i // XBD + 1))

        @block.sync
        def _(sync):
            for i, (gs, cnt) in enumerate(plan):
                if i == min(1, NDMA - 1):
                    sync.dma_start(out=iota2_sb.ap(),
                                   in_=iota2_h.ap()).then_inc(sem_aux, 16)
                    sync.dma_start(out=lab_sb.ap(),
                                   in_=lab_h.ap()).then_inc(sem_lab, 16)
                if i >= XBD:
                    pgs, pcnt = plan[i - XBD]
                    sync.wait_ge(sem_pe, pgs + pcnt)
                    sync.wait_ge(sem_sd, i - XBD + 1)
                    sync.wait_ge(sem_sa, i - XBD + 1)
                base = (i % XBD) * 2 * GCOLS
                if cnt == 2:
                    src = feat_h.ap()[gs:gs + 2].rearrange("g p c -> p g c")
                    dst = x_sb.ap()[:, base:base + 2 * GCOLS]
                else:
                    src = feat_h.ap()[gs]
                    dst = x_sb.ap()[:, base:base + GCOLS]
                sync.dma_start(out=dst, in_=src).then_inc(sem_xs[i % XBD], 16)
            # outputs: srow is ready once the square engines finished
            sync.wait_ge(sem_sd, NPLAN)
            sync.wait_ge(sem_sa, NPLAN)
            sync.dma_start(out=srow_h.ap(), in_=srow_sb.ap()).then_inc(
                sem_out, 16)
            sync.wait_ge(sem_cp, 1)
            sync.dma_start(out=out_h.ap()[:, D:D + P],
                           in_=out_sb.ap()[:, D:D + P]).then_inc(sem_out, 16)
            sync.wait_ge(sem_cp, 2)
            sync.dma_start(out=out_h.ap()[:, 0:D],
                           in_=out_sb.ap()[:, 0:D]).then_inc(sem_out, 16)
            sync.wait_ge(sem_out, 48)

        @block.vector
        def _(vector):
            if CC == CB:
                vector.memset(
                    srow_sb.ap().rearrange("p (g three) -> p g three",
                                           three=3)[:, :, 2:3], 0.0)
            vector.wait_ge(sem_aux, 16)
            vector.wait_ge(sem_lab, 16)
            with nc.allow_low_precision("bf16/fp8 row partials; err ~1e-4"):
                for i, (gs, cnt) in enumerate(plan):
                    wait_x(vector, gs)
                    for j in range(cnt):
                        g = gs + j
                        if g >= OHB:
                            vector.wait_ge(sem_pe, g - OHB + 1)
                        if "nooh" not in DBG:
                            vector.tensor_scalar(
                                oh_ap(g), iota2_sb.ap(),
                                lab_sb.ap()[:, g:g + 1], None,
                                mybir.AluOpType.is_equal,
                            ).then_inc(sem_oh, 1)
                        else:
                            vector.memset(oh_ap(g), 0.0).then_inc(sem_oh, 1)
                    if CB > CA and "nodve" not in DBG:
                        if i >= 3:
                            vector.wait_ge(sem_sd, i - 2)
                        w = CB - CA
                        sd = (i % 3) * 2 * w
                        so = scr_d.ap()[:, sd:sd + cnt * w].rearrange(
                            "p (c w) -> p c w", c=cnt)
                        vector.scalar_tensor_tensor(
                            out=so,
                            in0=ent_ap(i, cnt, CA, CB), scalar=1.0,
                            in1=ent_ap(i, cnt, CA, CB),
                            op0=mybir.AluOpType.mult,
                            op1=mybir.AluOpType.mult,
                            accum_out=srow_sb.ap()[:, 3 * i:3 * i + 1],
                        ).then_inc(sem_sd, 1)
                    else:
                        vector.memset(
                            srow_sb.ap()[:, 3 * i:3 * i + 1], 0.0
                        ).then_inc(sem_sd, 1)
                # tail: copy psum halves out
                # all PSUM reads stay on DVE: concurrent PSUM reads from
                # DVE + ACT on the same bank can crash the device
                vector.wait_ge(sem_pe, G)
                if NB > 0:
                    vector.tensor_copy(
                        out=out_sb.ap()[:, D:D + P],
                        in_=psum_g.ap()).then_inc(sem_cp, 1)
                else:
                    vector.memset(out_sb.ap()[:, D:D + P],
                                  0.0).then_inc(sem_cp, 1)
                vector.wait_ge(sem_seg, 1)
                vector.tensor_copy(out=out_sb.ap()[:, 0:D],
                                   in_=psum_s.ap()).then_inc(sem_cp, 1)

        @block.scalar
        def _(scalar):
            scalar.wait_ge(sem_aux, 16)
            with nc.allow_low_precision("bf16/fp8 row partials; err ~1e-4"):
                # dummy to trigger the ACT table load during the first DMA
                if "nodummy" not in DBG:
                    scalar.activation(
                        scr_t.ap(), iota2_sb.ap()[:, 0:16],
                        mybir.ActivationFunctionType.Square,
                    )
                for i, (gs, cnt) in enumerate(plan):
                    wait_x(scalar, gs)
                    if CA > 0 and "noact" not in DBG:
                        if i >= 3:
                            scalar.wait_ge(sem_sa, i - 2)
                        sa = (i % 3) * 2 * CA
                        so = scr_a.ap()[:, sa:sa + cnt * CA].rearrange(
                            "p (c w) -> p c w", c=cnt)
                        scalar.activation(
                            so, ent_ap(i, cnt, 0, CA),
                            mybir.ActivationFunctionType.Square,
                            accum_out=srow_sb.ap()[:, 3 * i + 1:3 * i + 2],
                        ).then_inc(sem_sa, 1)
                    else:
                        scalar.memzero(
                            srow_sb.ap()[:, 3 * i + 1:3 * i + 2]
                        ).then_inc(sem_sa, 1)

        if CC > CB:
            @block.gpsimd
            def _(gpsimd):
                with nc.allow_low_precision("bf16/fp8 row partials"):
                    for g in range(G):
                        wait_x(gpsimd, g)
                        if g >= 3:
                            gpsimd.wait_ge(sem_sg, g - 2)
                        sg = (g % 3) * (CC - CB)
                        xg = xg_ap(g)
                        gpsimd.scalar_tensor_tensor(
                            out=scr_g.ap()[:, sg:sg + CC - CB],
                            in0=xg[:, CB:CC], scalar=1.0,
                            in1=xg[:, CB:CC],
                            op0=mybir.AluOpType.mult,
                            op1=mybir.AluOpType.mult,
                            accum_out=srow_sb.ap()[:, 3 * g + 2:3 * g + 3],
                        ).then_inc(sem_sg, 1)
        @block.tensor
        def _(tensor):
            tensor.wait_ge(sem_aux, 16)
            # warm-up burst: junk matmuls to release the PE HAM clock gate
            for w in range(WARM):
                tensor.matmul(
                    out=psum_w.ap(), lhsT=iota2_sb.ap()[:, 0:P],
                    rhs=iota2_sb.ap()[:, P:2 * P], start=True, stop=True,
                )
            for g in range(G):
                tensor.wait_ge(sem_oh, g + 1)
                wait_x(tensor, g)
                xg = xg_ap(g)
                oh2 = oh_ap(g)
                if g == G - 1 and NB > 0:
                    for b in range(NB):
                        gm = tensor.matmul(
                            out=psum_g.ap(),
                            lhsT=xg[:, CC + b * P:CC + (b + 1) * P],
                            rhs=xg[:, CC + b * P:CC + (b + 1) * P],
                            start=(g == 0 and b == 0),
                            stop=(b == NB - 1),
                        )
                    gm.then_inc(sem_pe, 1)
                if USE_DR:
                    lhsT = oh2.rearrange("p (two c) -> p two c", two=2)
                    for t in range(GRP // 2):
                        rhs = xg[:, 2 * t * D:2 * (t + 1) * D].rearrange(
                            "p (two d) -> p two d", two=2)
                        last = tensor.matmul(
                            out=psum_s.ap(), lhsT=lhsT, rhs=rhs,
                            start=(g == 0 and t == 0),
                            stop=(g == G - 1 and t == GRP // 2 - 1),
                            perf_mode=mybir.MatmulPerfMode.DoubleRow,
                        )
                        if g == G - 1 and t == GRP // 2 - 1:
                            last.then_inc(sem_seg, 1)
                        if g == G - 1 and NB == 0 and t == GRP // 2 - 2:
                            last.then_inc(sem_pe, 1)
                else:
                    for t in range(GRP):
                        last = tensor.matmul(
                            out=psum_s.ap(), lhsT=oh2[:, 0:P],
                            rhs=xg[:, t * D:(t + 1) * D],
                            start=(g == 0 and t == 0),
                            stop=(g == G - 1 and t == GRP - 1),
                        )
                        if g == G - 1 and t == GRP - 1:
                            last.then_inc(sem_seg, 1)
                        if g == G - 1 and NB == 0 and t == GRP - 2:
                            last.then_inc(sem_pe, 1)
                if g < G - 1:
                    for b in range(NB):
                        blk = xg[:, CC + b * P:CC + (b + 1) * P]
                        last = tensor.matmul(
                            out=psum_g.ap(), lhsT=blk, rhs=blk,
                            start=(g == 0 and b == 0), stop=False,
                        )
                    last.then_inc(sem_pe, 1)

    nc.compile()
    return nc


def _finalize(results, labels: np.ndarray, C: int, N: int):
    sums = np.concatenate(
        [np.asarray(r["out"][:, :D], dtype=np.float64) for r in results],
        axis=0)  # [1024, D]
    ssq = 0.0
    for r in results:
        gram = np.asarray(r["out"][:, D:D + P], dtype=np.float64)
        ssq += float(np.trace(gram))
        srow = np.asarray(r["srow"], dtype=np.float64)
        srow3 = srow.reshape(srow.shape[0], -1, 3)
        if CB > CA:
            ssq += float(srow3[:, :, 0].sum())
        if CA > 0:
            ssq += float(srow3[:, :, 1].sum())
        if CC > CB:
            ssq += float(srow3[:, :, 2].sum())
    counts = np.bincount(labels, minlength=CORES * P).astype(np.float64)

    sums = sums[:C]
    counts = counts[:C]
    means = sums / counts[:, None]
    g = sums.sum(axis=0) / N
    tr_sw = ssq - float(((sums * sums).sum(axis=1) / counts).sum())
    tr_sb = float(((means - g) ** 2).sum())
    return np.asarray(np.float32(tr_sw / tr_sb))


def run(features, labels, num_classes, trace=False):
    features = np.asarray(features, dtype=np.float32)
    labels = np.asarray(labels).astype(np.int64).ravel()
    C = int(num_classes)
    N = features.shape[0]
    assert C <= CORES * P, f"num_classes={C} exceeds {CORES * P}"

    if trace:
        _ensure_ntff_hook()
    in_maps, G = _host_shard(features, labels)
    nc = _build_raw(G)
    res = run_bass_kernel_spmd(nc, in_maps, list(range(CORES)), trace=trace)
    out = _finalize(res.results, labels, C, N)
    return out, res


def kernel(**inputs) -> np.ndarray:
    trace = os.environ.get("KERNEL_TRACE", "0") == "1"
    out, _ = run(inputs["features"], inputs["labels"], inputs["num_classes"],
                 trace=trace)
    return out



# revision 7
# speedup vs baseline: 1.0007x; 1.0007x over previous
# Neural-collapse regularizer (tr_SW / tr_SB) on 8 TRN2 NeuronCores.
#
# Math: with per-class sums S_c = sum_{i: l_i=c} x_i, counts n_c,
# ssq = sum_i ||x_i||^2:
#   tr_SW = ssq - sum_c ||S_c||^2 / n_c
#   tr_SB = sum_c ||S_c/n_c - g||^2,  g = (sum_c S_c) / N
# The device computes the segment sums [128, D] per core plus ssq;
# everything else is tiny O(C*D) host math.
#
# Sharding: class-parallel. Core k owns classes [128k, 128(k+1)); the host
# routes each row to the core that owns its label.
#
# Layout: rows are packed in chunks of GRP=8 rows of a single class, one
# chunk per (group, partition) slot. Features are fp8 (e4m3) to cut HBM
# traffic 4x. The DRAM image is partition-major [128, G*LINE] where each
# group's per-partition line is OH2 (duplicated one-hot of the slot's
# class, 256 B) followed by the 8 row tiles (4096 B). Adjacent groups are
# contiguous per partition, so paired transfers move 8.7 KB descriptors.
#
# Segment sums: DoubleRow fp8 matmuls — each MM consumes a PAIR of row
# tiles (contraction 2x128) against the in-stream OH2 [128, 2, 128],
# accumulating into one PSUM bank across all groups.
#
# ssq: per group's [128, 4096] tile region, columns split three ways:
#   ACT:  Square + accum_out on cols [0, CA)
#   DVE:  scalar_tensor_tensor square + accum_out on cols [CA, CB)
#   PE:   "gram" matmuls B^T B on 128-col blocks of [CC, 4096), DoubleRow
#         pairs (two blocks per matmul), accumulated into a [128,128] PSUM
#         across the whole run; host takes the trace.
# Row partials land in a bf16 srow output; host sums them.

import contextlib
import ctypes
import os
import sys
import types

import numpy as np
import ml_dtypes

import concourse.bass as bass
import concourse.bacc as bacc
import concourse.mybir as mybir
from concourse.bass_utils import run_bass_kernel_spmd


def _ensure_ntff_hook():
    """Provide antenv.axon_hooks + an NTFF profile hook when the image's
    antenv package lacks it (needed only for trace=True timing runs)."""
    try:
        from antenv.axon_hooks import get_axon_ntff_profile_hook  # noqa: F401
        return
    except ImportError:
        pass
    mod = types.ModuleType("antenv.axon_hooks")
    state = {"hook": None}
    mod.set_axon_ntff_profile_hook = lambda h: state.__setitem__("hook", h)
    mod.get_axon_ntff_profile_hook = lambda: state["hook"]
    sys.modules["antenv.axon_hooks"] = mod

    so_path = "/opt/axon/libaxon_pjrt.so"
    if not os.path.exists(so_path):
        return
    lib = ctypes.CDLL(so_path)
    if not hasattr(lib, "axon_start_nrt_profile"):
        return
    lib.axon_start_nrt_profile.argtypes = [
        ctypes.POINTER(ctypes.c_int64), ctypes.c_size_t]
    lib.axon_start_nrt_profile.restype = ctypes.c_int64
    lib.axon_stop_nrt_profile.argtypes = [ctypes.c_char_p]
    lib.axon_stop_nrt_profile.restype = ctypes.c_int64

    @contextlib.contextmanager
    def _hook(output_dir, device_ids):
        import jax
        jax.devices()
        if device_ids:
            ids = (ctypes.c_int64 * len(device_ids))(*device_ids)
            rc = lib.axon_start_nrt_profile(ids, len(device_ids))
        else:
            rc = lib.axon_start_nrt_profile(None, 0)
        if rc != 0:
            raise RuntimeError(f"axon_start_nrt_profile rc={rc}")
        try:
            yield
        finally:
            n = lib.axon_stop_nrt_profile(str(output_dir).encode())
            print(f"profile: {n} file(s) written to {output_dir}",
                  file=sys.stderr)

    mod.set_axon_ntff_profile_hook(_hook)


CORES = 8
P = 128              # partitions = classes per core
D = 512              # feature dim
GRP = 8              # row-tiles per group = rows per chunk
GCOLS = GRP * D      # 4096 fp8 bytes of row data per partition per group
OHW = 2 * P          # 256 B in-stream duplicated one-hot
LINE = OHW + GCOLS   # 4352 B per group per partition
FP8 = mybir.dt.float8e4
BF16 = mybir.dt.bfloat16
F32 = mybir.dt.float32
NP_FP8 = ml_dtypes.float8_e4m3fn
NP_BF16 = ml_dtypes.bfloat16

# Column split of each group's [128, 4096] tile region for the ssq work.
CA = int(os.environ.get("K_CA", "1280"))    # ACT: cols [0, CA)
CB = int(os.environ.get("K_CB", "2432"))    # DVE: cols [CA, CB)
CC = int(os.environ.get("K_CC", "2432"))    # PE grams: cols [CC, 4096)
WARM = int(os.environ.get("K_WARM", "22"))  # PE warm-up matmuls
XBD = int(os.environ.get("K_XBD", "6"))     # double-group x buffers
USE_DR = os.environ.get("K_DR", "1") == "1"
USE_GDR = os.environ.get("K_GDR", "1") == "1"   # DoubleRow gram matmuls
DBG = set(os.environ.get("K_DBG", "").split(","))
assert CA % 128 == 0 and CB % 128 == 0 and CC % 128 == 0
assert CA <= CB <= CC <= GCOLS
NB = (GCOLS - CC) // 128                    # gram blocks per group


def _host_shard(features: np.ndarray, labels: np.ndarray):
    """Chunked class-sorted fp8 layout, partition-major with in-stream
    one-hots.

    Returns (in_maps, G). in_maps[k]:
      feat: [128, G*LINE] fp8 -- per partition, G lines of
            [oh2 (256B) | 8 row tiles (4096B)]
    """
    N, d = features.shape
    assert d == D, f"expected D={D}, got {d}"
    CPAD = CORES * P

    order = np.argsort(labels, kind="stable")
    sl = labels[order]
    class_start = np.searchsorted(sl, np.arange(CPAD + 1))
    counts = np.diff(class_start)
    chunks_per_class = -(-counts // GRP)
    core_chunks = chunks_per_class.reshape(CORES, P)
    G = int(-(-core_chunks.sum(axis=1).max() // P))

    f8 = features.astype(NP_FP8)
    eye2 = np.concatenate([np.eye(P, dtype=NP_FP8)] * 2, axis=1)  # [P, 256]

    in_maps = []
    for k in range(CORES):
        nch = core_chunks[k]
        total = int(nch.sum())
        assert total <= G * P
        chunk_cls = np.repeat(np.arange(P), nch)
        grid = np.full((G * P, GRP), -1, dtype=np.int64)
        cls_pad_start = np.concatenate(([0], np.cumsum(nch * GRP)))
        cnts = counts[k * P:(k + 1) * P]
        lo = class_start[k * P]
        n_k = int(cnts.sum())
        rows_k = order[lo:lo + n_k]
        within = np.arange(n_k) - np.repeat(class_start[k * P:(k + 1) * P] - lo,
                                            cnts)
        pos = np.repeat(cls_pad_start[:-1], cnts) + within
        grid.reshape(-1)[pos] = rows_k

        safe = np.maximum(grid, 0)
        fr = f8[safe.reshape(-1)]
        fr[grid.reshape(-1) < 0] = 0
        rows = fr.reshape(G, P, GCOLS)

        labg = np.zeros((G * P,), dtype=np.int64)
        labg[:total] = chunk_cls
        oh2 = eye2[labg].reshape(G, P, OHW)  # [G, P, 256]
        # empty slots have rows == 0 so their (class 0) one-hot is harmless

        feat = np.empty((G, P, LINE), dtype=NP_FP8)
        feat[:, :, :OHW] = oh2
        feat[:, :, OHW:] = rows
        feat = np.ascontiguousarray(
            feat.transpose(1, 0, 2).reshape(P, G * LINE))
        in_maps.append({"feat": feat})
    return in_maps, G


def _build_raw(G: int):
    # DMA plan: first three groups land as singles (fast pipeline fill),
    # then pairs; each plan entry owns one rotating double-width slot.
    plan = []
    g0 = 0
    while g0 < G:
        cnt = 1 if (g0 < 3 or g0 >= G - 2) else min(2, G - g0)
        plan.append((g0, cnt))
        g0 += cnt
    NPLAN = len(plan)
    _g2idx = {}
    for _i, (_gs, _cnt) in enumerate(plan):
        for _j in range(_cnt):
            _g2idx[_gs + _j] = (_i, _j)

    # srow layout: DVE partial per entry at col i, ACT at NPLAN+i.
    SROW = 2 * NPLAN

    nc = bacc.Bacc("TRN2", target_bir_lowering=False, debug=False,
                   enable_asserts=False)
    feat_h = nc.dram_tensor("feat", [P, G * LINE], FP8, kind="ExternalInput")
    out_h = nc.dram_tensor("out", [P, D + P], F32, kind="ExternalOutput")
    srow_h = nc.dram_tensor("srow", [P, SROW], BF16, kind="ExternalOutput")

    x_sb = nc.alloc_sbuf_tensor("x_sb", [P, XBD * 2 * LINE], FP8)
    scr_d = nc.alloc_sbuf_tensor("scr_d",
                                 [P, 3 * 2 * max(CB - CA, 16)], FP8)
    scr_a = nc.alloc_sbuf_tensor("scr_a", [P, 3 * 2 * max(CA, 16)], FP8)
    scr_t = nc.alloc_sbuf_tensor("scr_t", [P, 16], FP8)
    warm_sb = nc.alloc_sbuf_tensor("warm_sb", [1, 4096], FP8)
    srow_sb = nc.alloc_sbuf_tensor("srow_sb", [P, SROW], BF16)
    out_sb = nc.alloc_sbuf_tensor("out_sb", [P, D + P], F32)
    psum_s = nc.alloc_psum_tensor("psum_s", [P, D], F32)
    psum_g = nc.alloc_psum_tensor("psum_g", [P, P], F32)
    psum_w = nc.alloc_psum_tensor("psum_w", [P, P], F32)

    def plan_idx(g):
        return _g2idx[g][0]

    def ent_ap(i, cnt, lo, hi):
        # 3D slice [128, cnt, hi-lo] of plan entry i's tile columns
        base = (i % XBD) * 2 * LINE
        ap3 = x_sb.ap()[:, base:base + cnt * LINE].rearrange(
            "p (c w) -> p c w", c=cnt)
        return ap3[:, :, OHW + lo:OHW + hi]

    def xg_ap(g):
        # [128, LINE] view of group g: [oh2 | tiles]
        i, sub = _g2idx[g]
        off = (i % XBD) * 2 * LINE + sub * LINE
        return x_sb.ap()[:, off:off + LINE]

    import contextlib as _ctx
    with (
        _ctx.ExitStack() as _sems,
        nc.semaphore("sem_warm") as sem_warm,
        nc.semaphore("sem_sd") as sem_sd,
        nc.semaphore("sem_sa") as sem_sa,
        nc.semaphore("sem_pe") as sem_pe,
        nc.semaphore("sem_gram") as sem_gram,
        nc.semaphore("sem_cpg") as sem_cpg,
        nc.semaphore("sem_cps") as sem_cps,
        nc.semaphore("sem_seg") as sem_seg,
        nc.semaphore("sem_out") as sem_out,
        nc.Block() as block,
    ):
        sem_xs = [_sems.enter_context(nc.semaphore(f"sem_x{b}"))
                  for b in range(XBD)]

        def wait_x(eng, g):
            i = plan_idx(g)
            eng.wait_ge(sem_xs[i % XBD], 16 * (i // XBD + 1))

        @block.sync
        def _(sync):
            # tiny single-descriptor transfer whose completion (~data
            # arrival minus one entry) triggers the PE warm-up burst
            sync.dma_start(out=warm_sb.ap(),
                           in_=feat_h.ap()[0:1, 0:4096]).then_inc(
                sem_warm, 16)
            for i, (gs, cnt) in enumerate(plan):
                if i >= XBD:
                    pgs, pcnt = plan[i - XBD]
                    sync.wait_ge(sem_pe, pgs + pcnt)
                    sync.wait_ge(sem_sd, i - XBD + 1)
                    sync.wait_ge(sem_sa, i - XBD + 1)
                base = (i % XBD) * 2 * LINE
                src = feat_h.ap()[:, gs * LINE:(gs + cnt) * LINE]
                dst = x_sb.ap()[:, base:base + cnt * LINE]
                sync.dma_start(out=dst, in_=src).then_inc(sem_xs[i % XBD], 16)
            # outputs. gram psum copy lands first (grams run before segs in
            # the last group); then srow; then the seg-sum psum copy.
            sync.wait_ge(sem_cpg, 1)
            sync.dma_start(out=out_h.ap()[:, D:D + P],
                           in_=out_sb.ap()[:, D:D + P]).then_inc(sem_out, 16)
            sync.wait_ge(sem_sd, NPLAN)
            sync.wait_ge(sem_sa, NPLAN)
            sync.dma_start(out=srow_h.ap(), in_=srow_sb.ap()).then_inc(
                sem_out, 16)
            sync.wait_ge(sem_cps, 1)
            sync.dma_start(out=out_h.ap()[:, 0:D],
                           in_=out_sb.ap()[:, 0:D]).then_inc(sem_out, 16)
            sync.wait_ge(sem_out, 48)

        @block.vector
        def _(vector):
            with nc.allow_low_precision("bf16/fp8 row partials; err ~1e-4"):
                for i, (gs, cnt) in enumerate(plan):
                    wait_x(vector, gs)
                    if CB > CA and "nodve" not in DBG:
                        if i >= 3:
                            vector.wait_ge(sem_sd, i - 2)
                        w = CB - CA
                        sd = (i % 3) * 2 * w
                        so = scr_d.ap()[:, sd:sd + cnt * w].rearrange(
                            "p (c w) -> p c w", c=cnt)
                        vector.scalar_tensor_tensor(
                            out=so,
                            in0=ent_ap(i, cnt, CA, CB), scalar=1.0,
                            in1=ent_ap(i, cnt, CA, CB),
                            op0=mybir.AluOpType.mult,
                            op1=mybir.AluOpType.mult,
                            accum_out=srow_sb.ap()[:, i:i + 1],
                        ).then_inc(sem_sd, 1)
                    else:
                        vector.memset(
                            srow_sb.ap()[:, i:i + 1], 0.0
                        ).then_inc(sem_sd, 1)
                # tail: copy the seg-sum psum half (ACT handles the gram
                # half concurrently; different PSUM banks)
                vector.wait_ge(sem_seg, 1)
                vector.tensor_copy(out=out_sb.ap()[:, 0:D],
                                   in_=psum_s.ap()).then_inc(sem_cps, 1)

        @block.scalar
        def _(scalar):
            with nc.allow_low_precision("bf16/fp8 row partials; err ~1e-4"):
                # dummy to trigger the ACT table load immediately (reads
                # whatever is in SBUF; result is scratch)
                if "nodummy" not in DBG:
                    scalar.activation(
                        scr_t.ap(), scr_a.ap()[:, 0:16],
                        mybir.ActivationFunctionType.Square,
                    )
                for i, (gs, cnt) in enumerate(plan):
                    wait_x(scalar, gs)
                    if CA > 0 and "noact" not in DBG:
                        if i >= 3:
                            scalar.wait_ge(sem_sa, i - 2)
                        sa = (i % 3) * 2 * CA
                        so = scr_a.ap()[:, sa:sa + cnt * CA].rearrange(
                            "p (c w) -> p c w", c=cnt)
                        scalar.activation(
                            so, ent_ap(i, cnt, 0, CA),
                            mybir.ActivationFunctionType.Square,
                            accum_out=srow_sb.ap()[:, NPLAN + i:NPLAN + i + 1],
                        ).then_inc(sem_sa, 1)
                    else:
                        scalar.memzero(
                            srow_sb.ap()[:, NPLAN + i:NPLAN + i + 1]
                        ).then_inc(sem_sa, 1)
                # tail: copy gram psum to sbuf (concurrent with DVE's
                # psum_s copy; different banks so no DVE/ACT conflict)
                if NB > 0:
                    scalar.wait_ge(sem_gram, 1)
                    scalar.activation(
                        out_sb.ap()[:, D:D + P], psum_g.ap(),
                        mybir.ActivationFunctionType.Copy,
                    ).then_inc(sem_cpg, 1)
                else:
                    scalar.memzero(out_sb.ap()[:, D:D + P]).then_inc(
                        sem_cpg, 1)

        @block.tensor
        def _(tensor):
            # warm-up burst: junk matmuls to release the PE HAM clock gate.
            # Times itself against the 1-descriptor warm DMA: done right
            # when the first group's data becomes available.
            tensor.wait_ge(sem_warm, 16)
            for w in range(WARM):
                tensor.matmul(
                    out=psum_w.ap(), lhsT=scr_a.ap()[:, 0:P],
                    rhs=scr_a.ap()[:, P:2 * P], start=True, stop=True,
                )
            for g in range(G):
                wait_x(tensor, g)
                xg = xg_ap(g)
                tiles = xg[:, OHW:]
                # grams first: the final gram (plus its psum copy + DMA)
                # overlaps the last seg matmuls
                if NB > 0:
                    if USE_GDR:
                        gi = 0
                        for b in range(NB // 2):
                            blk2 = tiles[:, CC + 2 * b * P:
                                         CC + 2 * (b + 1) * P].rearrange(
                                "p (two c) -> p two c", two=2)
                            gm = tensor.matmul(
                                out=psum_g.ap(), lhsT=blk2, rhs=blk2,
                                start=(g == 0 and gi == 0),
                                stop=(g == G - 1 and NB % 2 == 0
                                      and b == NB // 2 - 1),
                                perf_mode=mybir.MatmulPerfMode.DoubleRow,
                            )
                            gi += 1
                        if NB % 2:
                            blk = tiles[:, CC + (NB - 1) * P:CC + NB * P]
                            gm = tensor.matmul(
                                out=psum_g.ap(), lhsT=blk, rhs=blk,
                                start=(g == 0 and gi == 0),
                                stop=(g == G - 1),
                            )
                    else:
                        for b in range(NB):
                            blk = tiles[:, CC + b * P:CC + (b + 1) * P]
                            gm = tensor.matmul(
                                out=psum_g.ap(), lhsT=blk, rhs=blk,
                                start=(g == 0 and b == 0),
                                stop=(g == G - 1 and b == NB - 1),
                            )
                    if g == G - 1:
                        gm.then_inc(sem_gram, 1)
                mms = []
                if USE_DR:
                    lhsT = xg[:, 0:OHW].rearrange("p (two c) -> p two c",
                                                  two=2)
                    for t in range(GRP // 2):
                        mms.append(tensor.matmul(
                            out=psum_s.ap(), lhsT=lhsT, rhs=tiles[
                                :, 2 * t * D:2 * (t + 1) * D].rearrange(
                                "p (two d) -> p two d", two=2),
                            start=(g == 0 and t == 0),
                            stop=(g == G - 1 and t == GRP // 2 - 1),
                            perf_mode=mybir.MatmulPerfMode.DoubleRow,
                        ))
                else:
                    for t in range(GRP):
                        mms.append(tensor.matmul(
                            out=psum_s.ap(), lhsT=xg[:, 0:P],
                            rhs=tiles[:, t * D:(t + 1) * D],
                            start=(g == 0 and t == 0),
                            stop=(g == G - 1 and t == GRP - 1),
                        ))
                # an instruction carries at most one sem update: put
                # sem_seg on the last matmul, sem_pe on the previous one
                if g == G - 1:
                    mms[-1].then_inc(sem_seg, 1)
                    mms[-2].then_inc(sem_pe, 1)
                else:
                    mms[-1].then_inc(sem_pe, 1)

    nc.compile()
    return nc


def _finalize(results, labels: np.ndarray, C: int, N: int):
    sums = np.concatenate(
        [np.asarray(r["out"][:, :D], dtype=np.float64) for r in results],
        axis=0)  # [1024, D]
    ssq = 0.0
    for r in results:
        gram = np.asarray(r["out"][:, D:D + P], dtype=np.float64)
        ssq += float(np.trace(gram))
        ssq += float(np.asarray(r["srow"], dtype=np.float64).sum())
    counts = np.bincount(labels, minlength=CORES * P).astype(np.float64)

    sums = sums[:C]
    counts = counts[:C]
    means = sums / counts[:, None]
    g = sums.sum(axis=0) / N
    tr_sw = ssq - float(((sums * sums).sum(axis=1) / counts).sum())
    tr_sb = float(((means - g) ** 2).sum())
    return np.asarray(np.float32(tr_sw / tr_sb))


def run(features, labels, num_classes, trace=False):
    features = np.asarray(features, dtype=np.float32)
    labels = np.asarray(labels).astype(np.int64).ravel()
    C = int(num_classes)
    N = features.shape[0]
    assert C <= CORES * P, f"num_classes={C} exceeds {CORES * P}"

    if trace:
        _ensure_ntff_hook()
    in_maps, G = _host_shard(features, labels)
    nc = _build_raw(G)
    res = run_bass_kernel_spmd(nc, in_maps, list(range(CORES)), trace=trace)
    out = _finalize(res.results, labels, C, N)
    return out, res


def kernel(**inputs) -> np.ndarray:
    trace = os.environ.get("KERNEL_TRACE", "0") == "1"
    out, _ = run(inputs["features"], inputs["labels"], inputs["num_classes"],
                 trace=trace)
    return out


# revision 15
# speedup vs baseline: 1.0706x; 1.0698x over previous
# Neural-collapse regularizer (tr_SW / tr_SB) on 8 TRN2 NeuronCores.
#
# Math: with per-class sums S_c = sum_{i: l_i=c} x_i, counts n_c,
# ssq = sum_i ||x_i||^2:
#   tr_SW = ssq - sum_c ||S_c||^2 / n_c
#   tr_SB = sum_c ||S_c/n_c - g||^2,  g = (sum_c S_c) / N
# The device computes the segment sums [128, D] per core plus ssq;
# everything else is tiny O(C*D) host math.
#
# Sharding: class-parallel. Core k owns classes [128k, 128(k+1)); the host
# routes each row to the core that owns its label.
#
# Layout: rows are packed in chunks of GRP=8 rows of a single class, one
# chunk per (group, partition) slot. Features are fp8 (e4m3) to cut HBM
# traffic 4x. The DRAM image is partition-major [128, G*LINE] where each
# group's per-partition line is OH2 (duplicated one-hot of the slot's
# class, 256 B) followed by the 8 row tiles (4096 B). Adjacent groups are
# contiguous per partition, so paired transfers move 8.7 KB descriptors.
#
# Segment sums: DoubleRow fp8 matmuls — each MM consumes a PAIR of row
# tiles (contraction 2x128) against the in-stream OH2 [128, 2, 128],
# accumulating into one PSUM bank across all groups.
#
# ssq: per group's [128, 4096] tile region, columns split three ways:
#   ACT:  Square + accum_out on cols [0, CA)
#   DVE:  scalar_tensor_tensor square + accum_out on cols [CA, CB)
#   PE:   "gram" matmuls B^T B on 128-col blocks of [CC, 4096), DoubleRow
#         pairs (two blocks per matmul), accumulated into a [128,128] PSUM
#         across the whole run; host takes the trace.
# Row partials land in a bf16 srow output; host sums them.

import contextlib
import ctypes
import os
import sys
import types

import numpy as np
import ml_dtypes

import concourse.bass as bass
import concourse.bacc as bacc
import concourse.mybir as mybir
from concourse.bass_utils import run_bass_kernel_spmd


def _ensure_ntff_hook():
    """Provide antenv.axon_hooks + an NTFF profile hook when the image's
    antenv package lacks it (needed only for trace=True timing runs)."""
    try:
        from antenv.axon_hooks import get_axon_ntff_profile_hook  # noqa: F401
        return
    except ImportError:
        pass
    mod = types.ModuleType("antenv.axon_hooks")
    state = {"hook": None}
    mod.set_axon_ntff_profile_hook = lambda h: state.__setitem__("hook", h)
    mod.get_axon_ntff_profile_hook = lambda: state["hook"]
    sys.modules["antenv.axon_hooks"] = mod

    so_path = "/opt/axon/libaxon_pjrt.so"
    if not os.path.exists(so_path):
        return
    lib = ctypes.CDLL(so_path)
    if not hasattr(lib, "axon_start_nrt_profile"):
        return
    lib.axon_start_nrt_profile.argtypes = [
        ctypes.POINTER(ctypes.c_int64), ctypes.c_size_t]
    lib.axon_start_nrt_profile.restype = ctypes.c_int64
    lib.axon_stop_nrt_profile.argtypes = [ctypes.c_char_p]
    lib.axon_stop_nrt_profile.restype = ctypes.c_int64

    @contextlib.contextmanager
    def _hook(output_dir, device_ids):
        import jax
        jax.devices()
        if device_ids:
            ids = (ctypes.c_int64 * len(device_ids))(*device_ids)
            rc = lib.axon_start_nrt_profile(ids, len(device_ids))
        else:
            rc = lib.axon_start_nrt_profile(None, 0)
        if rc != 0:
            raise RuntimeError(f"axon_start_nrt_profile rc={rc}")
        try:
            yield
        finally:
            n = lib.axon_stop_nrt_profile(str(output_dir).encode())
            print(f"profile: {n} file(s) written to {output_dir}",
                  file=sys.stderr)

    mod.set_axon_ntff_profile_hook(_hook)


CORES = 8
P = 128              # partitions = classes per core
D = 512              # feature dim
GRP = 8              # row-tiles per group = rows per chunk
GCOLS = GRP * D      # 4096 fp8 bytes of row data per partition per group
OHW = 2 * P          # 256 B in-stream duplicated one-hot
LINE = OHW + GCOLS   # 4352 B per group per partition
FP8 = mybir.dt.float8e4
BF16 = mybir.dt.bfloat16
F32 = mybir.dt.float32
NP_FP8 = ml_dtypes.float8_e4m3fn
NP_BF16 = ml_dtypes.bfloat16

# Column split of each group's [128, 4096] tile region for the ssq work.
CA = int(os.environ.get("K_CA", "1280"))    # ACT: cols [0, CA)
CB = int(os.environ.get("K_CB", "2432"))    # DVE: cols [CA, CB)
CC = int(os.environ.get("K_CC", "2432"))    # PE grams: cols [CC, 4096)
WARM = int(os.environ.get("K_WARM", "40"))  # PE warm-up matmuls
XBD = int(os.environ.get("K_XBD", "6"))     # double-group x buffers
USE_DR = os.environ.get("K_DR", "1") == "1"
USE_GDR = os.environ.get("K_GDR", "1") == "1"   # DoubleRow gram matmuls
DBG = set(os.environ.get("K_DBG", "").split(","))
assert CA % 128 == 0 and CB % 128 == 0 and CC % 128 == 0
assert CA <= CB <= CC <= GCOLS
NB = (GCOLS - CC) // 128                    # gram blocks per group


def _host_shard(features: np.ndarray, labels: np.ndarray):
    """Chunked class-sorted fp8 layout, partition-major with in-stream
    one-hots.

    Returns (in_maps, G). in_maps[k]:
      feat: [128, G*LINE] fp8 -- per partition, G lines of
            [oh2 (256B) | 8 row tiles (4096B)]
    """
    N, d = features.shape
    assert d == D, f"expected D={D}, got {d}"
    CPAD = CORES * P

    order = np.argsort(labels, kind="stable")
    sl = labels[order]
    class_start = np.searchsorted(sl, np.arange(CPAD + 1))
    counts = np.diff(class_start)
    chunks_per_class = -(-counts // GRP)
    core_chunks = chunks_per_class.reshape(CORES, P)
    G = int(-(-core_chunks.sum(axis=1).max() // P))

    f8 = features.astype(NP_FP8)
    eye2 = np.concatenate([np.eye(P, dtype=NP_FP8)] * 2, axis=1)  # [P, 256]

    in_maps = []
    for k in range(CORES):
        nch = core_chunks[k]
        total = int(nch.sum())
        assert total <= G * P
        chunk_cls = np.repeat(np.arange(P), nch)
        grid = np.full((G * P, GRP), -1, dtype=np.int64)
        cls_pad_start = np.concatenate(([0], np.cumsum(nch * GRP)))
        cnts = counts[k * P:(k + 1) * P]
        lo = class_start[k * P]
        n_k = int(cnts.sum())
        rows_k = order[lo:lo + n_k]
        within = np.arange(n_k) - np.repeat(class_start[k * P:(k + 1) * P] - lo,
                                            cnts)
        pos = np.repeat(cls_pad_start[:-1], cnts) + within
        grid.reshape(-1)[pos] = rows_k

        safe = np.maximum(grid, 0)
        fr = f8[safe.reshape(-1)]
        fr[grid.reshape(-1) < 0] = 0
        rows = fr.reshape(G, P, GCOLS)

        labg = np.zeros((G * P,), dtype=np.int64)
        labg[:total] = chunk_cls
        oh2 = eye2[labg].reshape(G, P, OHW)  # [G, P, 256]
        # empty slots have rows == 0 so their (class 0) one-hot is harmless

        feat = np.empty((G, P, LINE), dtype=NP_FP8)
        feat[:, :, :OHW] = oh2
        feat[:, :, OHW:] = rows
        feat = np.ascontiguousarray(
            feat.transpose(1, 0, 2).reshape(P, G * LINE))
        in_maps.append({"feat": feat})
    return in_maps, G


def _build_raw(G: int):
    # DMA plan: first three groups land as singles (fast pipeline fill),
    # then pairs; each plan entry owns one rotating double-width slot.
    plan = []
    g0 = 0
    while g0 < G:
        cnt = 1 if (g0 < 3 or g0 >= G - 2) else min(2, G - g0)
        plan.append((g0, cnt))
        g0 += cnt
    NPLAN = len(plan)
    _g2idx = {}
    for _i, (_gs, _cnt) in enumerate(plan):
        for _j in range(_cnt):
            _g2idx[_gs + _j] = (_i, _j)

    # out layout (all bf16): seg sums [0, D), gram trace [D, D+P),
    # DVE ssq partial per entry at D+P+i, ACT at D+P+NPLAN+i.
    OW = D + P + 2 * NPLAN

    nc = bacc.Bacc("TRN2", target_bir_lowering=False, debug=False,
                   enable_asserts=False)
    feat_h = nc.dram_tensor("feat", [P, G * LINE], FP8, kind="ExternalInput")
    out_h = nc.dram_tensor("out", [P, OW], BF16, kind="ExternalOutput")

    x_sb = nc.alloc_sbuf_tensor("x_sb", [P, XBD * 2 * LINE], FP8)
    scr_d = nc.alloc_sbuf_tensor("scr_d",
                                 [P, 3 * 2 * max(CB - CA, 16)], FP8)
    scr_a = nc.alloc_sbuf_tensor("scr_a", [P, 3 * 2 * max(CA, 16)], FP8)
    scr_t = nc.alloc_sbuf_tensor("scr_t", [P, 16], FP8)
    warm_sb = nc.alloc_sbuf_tensor("warm_sb", [16, 256], FP8)
    out_sb = nc.alloc_sbuf_tensor("out_sb", [P, OW], BF16)
    psum_s = nc.alloc_psum_tensor("psum_s", [P, D], F32)
    psum_g = nc.alloc_psum_tensor("psum_g", [P, P], F32)
    psum_w = nc.alloc_psum_tensor("psum_w", [P, P], F32)

    def plan_idx(g):
        return _g2idx[g][0]

    def ent_ap(i, cnt, lo, hi):
        # 3D slice [128, cnt, hi-lo] of plan entry i's tile columns
        base = (i % XBD) * 2 * LINE
        ap3 = x_sb.ap()[:, base:base + cnt * LINE].rearrange(
            "p (c w) -> p c w", c=cnt)
        return ap3[:, :, OHW + lo:OHW + hi]

    def xg_ap(g):
        # [128, LINE] view of group g: [oh2 | tiles]
        i, sub = _g2idx[g]
        off = (i % XBD) * 2 * LINE + sub * LINE
        return x_sb.ap()[:, off:off + LINE]

    import contextlib as _ctx
    with (
        _ctx.ExitStack() as _sems,
        nc.semaphore("sem_warm") as sem_warm,
        nc.semaphore("sem_sd") as sem_sd,
        nc.semaphore("sem_sa") as sem_sa,
        nc.semaphore("sem_pe") as sem_pe,
        nc.semaphore("sem_gram") as sem_gram,
        nc.semaphore("sem_cpg") as sem_cpg,
        nc.semaphore("sem_cps") as sem_cps,
        nc.semaphore("sem_seg") as sem_seg,
        nc.semaphore("sem_out") as sem_out,
        nc.Block() as block,
    ):
        sem_xs = [_sems.enter_context(nc.semaphore(f"sem_x{b}"))
                  for b in range(XBD)]

        def wait_x(eng, g):
            i = plan_idx(g)
            eng.wait_ge(sem_xs[i % XBD], 16 * (i // XBD + 1))

        @block.sync
        def _(sync):
            for i, (gs, cnt) in enumerate(plan):
                if i >= XBD:
                    pgs, pcnt = plan[i - XBD]
                    sync.wait_ge(sem_pe, pgs + pcnt)
                    sync.wait_ge(sem_sd, i - XBD + 1)
                    sync.wait_ge(sem_sa, i - XBD + 1)
                base = (i % XBD) * 2 * LINE
                src = feat_h.ap()[:, gs * LINE:(gs + cnt) * LINE]
                dst = x_sb.ap()[:, base:base + cnt * LINE]
                sync.dma_start(out=dst, in_=src).then_inc(sem_xs[i % XBD], 16)
            # single combined output DMA once every writer is done
            sync.wait_ge(sem_sd, NPLAN)
            sync.wait_ge(sem_sa, NPLAN)
            sync.wait_ge(sem_cpg, 1)
            sync.wait_ge(sem_cps, 1)
            sync.dma_start(out=out_h.ap(), in_=out_sb.ap()).then_inc(
                sem_out, 16)
            sync.wait_ge(sem_out, 16)

        @block.vector
        def _(vector):
            with nc.allow_low_precision("bf16/fp8 row partials; err ~1e-4"):
                for i, (gs, cnt) in enumerate(plan):
                    wait_x(vector, gs)
                    if CB > CA and "nodve" not in DBG:
                        if i >= 3:
                            vector.wait_ge(sem_sd, i - 2)
                        w = CB - CA
                        sd = (i % 3) * 2 * w
                        so = scr_d.ap()[:, sd:sd + cnt * w].rearrange(
                            "p (c w) -> p c w", c=cnt)
                        vector.scalar_tensor_tensor(
                            out=so,
                            in0=ent_ap(i, cnt, CA, CB), scalar=1.0,
                            in1=ent_ap(i, cnt, CA, CB),
                            op0=mybir.AluOpType.mult,
                            op1=mybir.AluOpType.mult,
                            accum_out=out_sb.ap()[:, D + P + i:D + P + i + 1],
                        ).then_inc(sem_sd, 1)
                    else:
                        vector.memset(
                            out_sb.ap()[:, D + P + i:D + P + i + 1], 0.0
                        ).then_inc(sem_sd, 1)
                # tail: copy the seg-sum psum half (ACT handles the gram
                # half concurrently; different PSUM banks)
                vector.wait_ge(sem_seg, 1)
                vector.tensor_copy(out=out_sb.ap()[:, 0:D],
                                   in_=psum_s.ap()).then_inc(sem_cps, 1)

        @block.scalar
        def _(scalar):
            # tiny 16-descriptor transfer, issued from the scalar engine's
            # queues while the sync engine is still in its prologue: its
            # completion triggers the PE warm-up burst early
            scalar.dma_start(out=warm_sb.ap(),
                             in_=feat_h.ap()[0:16, 0:256]).then_inc(
                sem_warm, 16)
            with nc.allow_low_precision("bf16/fp8 row partials; err ~1e-4"):
                # dummy to trigger the ACT table load immediately (reads
                # whatever is in SBUF; result is scratch)
                if "nodummy" not in DBG:
                    scalar.activation(
                        scr_t.ap(), scr_a.ap()[:, 0:16],
                        mybir.ActivationFunctionType.Square,
                    )
                for i, (gs, cnt) in enumerate(plan):
                    wait_x(scalar, gs)
                    if CA > 0 and "noact" not in DBG:
                        if i >= 3:
                            scalar.wait_ge(sem_sa, i - 2)
                        sa = (i % 3) * 2 * CA
                        so = scr_a.ap()[:, sa:sa + cnt * CA].rearrange(
                            "p (c w) -> p c w", c=cnt)
                        scalar.activation(
                            so, ent_ap(i, cnt, 0, CA),
                            mybir.ActivationFunctionType.Square,
                            accum_out=out_sb.ap()[
                                :, D + P + NPLAN + i:D + P + NPLAN + i + 1],
                        ).then_inc(sem_sa, 1)
                    else:
                        scalar.memzero(
                            out_sb.ap()[
                                :, D + P + NPLAN + i:D + P + NPLAN + i + 1]
                        ).then_inc(sem_sa, 1)
                # tail: copy gram psum to sbuf (concurrent with DVE's
                # psum_s copy; different banks so no DVE/ACT conflict)
                if NB > 0:
                    scalar.wait_ge(sem_gram, 1)
                    scalar.activation(
                        out_sb.ap()[:, D:D + P], psum_g.ap(),
                        mybir.ActivationFunctionType.Copy,
                    ).then_inc(sem_cpg, 1)
                else:
                    scalar.memzero(out_sb.ap()[:, D:D + P]).then_inc(
                        sem_cpg, 1)

        @block.tensor
        def _(tensor):
            # warm-up burst: junk matmuls to release the PE HAM clock gate.
            # Times itself against the 1-descriptor warm DMA: done right
            # when the first group's data becomes available.
            tensor.wait_ge(sem_warm, 16)
            for w in range(WARM):
                tensor.matmul(
                    out=psum_w.ap(), lhsT=scr_a.ap()[:, 0:P],
                    rhs=scr_a.ap()[:, P:2 * P], start=True, stop=True,
                )
            for g in range(G):
                wait_x(tensor, g)
                xg = xg_ap(g)
                tiles = xg[:, OHW:]
                # segs first so the final psum_s copy starts as early as
                # possible; grams follow (their copy overlaps the tail)
                mms = []
                if USE_DR:
                    lhsT = xg[:, 0:OHW].rearrange("p (two c) -> p two c",
                                                  two=2)
                    for t in range(GRP // 2):
                        mms.append(tensor.matmul(
                            out=psum_s.ap(), lhsT=lhsT, rhs=tiles[
                                :, 2 * t * D:2 * (t + 1) * D].rearrange(
                                "p (two d) -> p two d", two=2),
                            start=(g == 0 and t == 0),
                            stop=(g == G - 1 and t == GRP // 2 - 1),
                            perf_mode=mybir.MatmulPerfMode.DoubleRow,
                        ))
                else:
                    for t in range(GRP):
                        mms.append(tensor.matmul(
                            out=psum_s.ap(), lhsT=xg[:, 0:P],
                            rhs=tiles[:, t * D:(t + 1) * D],
                            start=(g == 0 and t == 0),
                            stop=(g == G - 1 and t == GRP - 1),
                        ))
                if g == G - 1:
                    mms[-1].then_inc(sem_seg, 1)
                if NB > 0:
                    gms = []
                    if USE_GDR:
                        for b in range(NB // 2):
                            blk2 = tiles[:, CC + 2 * b * P:
                                         CC + 2 * (b + 1) * P].rearrange(
                                "p (two c) -> p two c", two=2)
                            gms.append(tensor.matmul(
                                out=psum_g.ap(), lhsT=blk2, rhs=blk2,
                                start=(g == 0 and b == 0),
                                stop=(g == G - 1 and NB % 2 == 0
                                      and b == NB // 2 - 1),
                                perf_mode=mybir.MatmulPerfMode.DoubleRow,
                            ))
                        if NB % 2:
                            blk = tiles[:, CC + (NB - 1) * P:CC + NB * P]
                            gms.append(tensor.matmul(
                                out=psum_g.ap(), lhsT=blk, rhs=blk,
                                start=(g == 0 and NB // 2 == 0),
                                stop=(g == G - 1),
                            ))
                    else:
                        for b in range(NB):
                            blk = tiles[:, CC + b * P:CC + (b + 1) * P]
                            gms.append(tensor.matmul(
                                out=psum_g.ap(), lhsT=blk, rhs=blk,
                                start=(g == 0 and b == 0),
                                stop=(g == G - 1 and b == NB - 1),
                            ))
                    if g == G - 1:
                        gms[-1].then_inc(sem_gram, 1)
                    else:
                        gms[-1].then_inc(sem_pe, 1)
                else:
                    if g < G - 1:
                        mms[-1].then_inc(sem_pe, 1)

    nc.compile()
    return nc


def _finalize(results, labels: np.ndarray, C: int, N: int):
    sums = np.concatenate(
        [np.asarray(r["out"][:, :D], dtype=np.float64) for r in results],
        axis=0)  # [1024, D]
    ssq = 0.0
    for r in results:
        gram = np.asarray(r["out"][:, D:D + P], dtype=np.float64)
        ssq += float(np.trace(gram))
        ssq += float(np.asarray(r["out"][:, D + P:], dtype=np.float64).sum())
    counts = np.bincount(labels, minlength=CORES * P).astype(np.float64)

    sums = sums[:C]
    counts = counts[:C]
    means = sums / counts[:, None]
    g = sums.sum(axis=0) / N
    tr_sw = ssq - float(((sums * sums).sum(axis=1) / counts).sum())
    tr_sb = float(((means - g) ** 2).sum())
    return np.asarray(np.float32(tr_sw / tr_sb))


def run(features, labels, num_classes, trace=False):
    features = np.asarray(features, dtype=np.float32)
    labels = np.asarray(labels).astype(np.int64).ravel()
    C = int(num_classes)
    N = features.shape[0]
    assert C <= CORES * P, f"num_classes={C} exceeds {CORES * P}"

    if trace:
        _ensure_ntff_hook()
    in_maps, G = _host_shard(features, labels)
    nc = _build_raw(G)
    res = run_bass_kernel_spmd(nc, in_maps, list(range(CORES)), trace=trace)
    out = _finalize(res.results, labels, C, N)
    return out, res


def kernel(**inputs) -> np.ndarray:
    trace = os.environ.get("KERNEL_TRACE", "0") == "1"
    out, _ = run(inputs["features"], inputs["labels"], inputs["num_classes"],
                 trace=trace)
    return out
